# revision 1
# baseline (speedup 1.0000x reference)
"""Trainium2 Bass kernel for DeepSeek-V3-style block-sparse MoE MLP.

Strategy (expert-parallel across 8 NeuronCores):
  - Each core holds 4 of the 32 experts' weights (fp16) and computes the
    partial output sum over its local experts for ALL 256 tokens.
  - The small router gate is replicated: every core computes the full
    group-limited top-k routing on-device, then selects its local experts'
    routing weights via a per-core one-hot matrix (pure data, so the SPMD
    program is identical on every core).
  - All matmuls are fp16: same PE rate as bf16 but 10-bit mantissa, and
    the tiny weight/activation ranges cannot overflow. The router logits
    use a split-precision hi/lo fp16 decomposition, far below the
    routing decision margins. (fp32 matmuls are ruled out by a walrus
    codegen limit of one semaphore wait per self-loading instruction;
    see _spill_excess_waits.)
  - Routing weights are folded into the activations before the down
    projection, so the PSUM accumulation over (expert, i-chunk) directly
    yields the core's partial output. Host just sums the 8 partials.
"""
import sys
sys.path.insert(0, '/opt/trn_rl_repo')
import numpy as np
import ml_dtypes
import concourse.mybir as mybir
import concourse.tile as tile
from concourse import bass
from concourse.bass_utils import run_bass_kernel_spmd

T, H, I, E = 256, 1024, 512, 32
N_CORES = 8
E_LOC = E // N_CORES            # 4 experts per core
N_GROUP, GSZ = 8, 4             # 8 groups of 4 experts
ROUTED_SCALING_FACTOR = 2.5
P = 128
NTT = T // P                    # token tiles
NHC = H // P                    # h chunks (contraction for up/gate proj)
NIC = I // P                    # i chunks (contraction for down proj)
HH = H // 512                   # h halves for down-proj PSUM banks
dt = mybir.dt
F32, BF = dt.float32, dt.float16
Alu = mybir.AluOpType
Act = mybir.ActivationFunctionType

_CACHE = {}


def _build():
    nc = bass.Bass('TRN2')
    # all inputs are host-pre-shuffled to partition-major layouts so every
    # DMA reads long contiguous runs per partition (8 KB for weights)
    xtb_d = nc.dram_tensor('xtb', [P, NHC * T], BF, kind='ExternalInput')
    xtlo_d = nc.dram_tensor('xtlo', [P, NHC * T], BF, kind='ExternalInput')
    gcat_d = nc.dram_tensor('gcat', [P, NHC * 2 * E], BF, kind='ExternalInput')
    biasb_d = nc.dram_tensor('biasb', [P, E], F32, kind='ExternalInput')
    selbc_d = nc.dram_tensor('selbc', [E, E_LOC * P], BF, kind='ExternalInput')
    # wg/wu: [p, e, s, c, i'] with i = s*256 + i'; wd: [p, e, c, h]
    wg_d = nc.dram_tensor('wg', [P, E_LOC * 2 * NHC * 256], BF, kind='ExternalInput')
    wu_d = nc.dram_tensor('wu', [P, E_LOC * 2 * NHC * 256], BF, kind='ExternalInput')
    wd_d = nc.dram_tensor('wd', [P, E_LOC * NIC * H], BF, kind='ExternalInput')
    out_d = nc.dram_tensor('out', [T, H], BF, kind='ExternalOutput')

    with tile.TileContext(nc) as tc:
        with tc.tile_pool(name='consts', bufs=1) as consts, \
             tc.tile_pool(name='wpool', bufs=1) as wpool, \
             tc.tile_pool(name='rt', bufs=2) as rt, \
             tc.tile_pool(name='actp', bufs=4) as actp, \
             tc.tile_pool(name='atp', bufs=1) as atp, \
             tc.tile_pool(name='outp', bufs=1) as outp, \
             tc.tile_pool(name='ps', bufs=1, space='PSUM') as ps, \
             tc.tile_pool(name='psy', bufs=1, space='PSUM') as psy:

            # ---------- PE warmup (fills the DMA head, beats HAM cold) ----
            scratch_bf = consts.tile([P, 512], BF)
            nc.vector.memset(scratch_bf, 0.0)
            pwarm = ps.tile([P, 512], F32, name='pwarm', tag='ps_small', bufs=2)
            N_WARM = 10
            for i in range(N_WARM):
                nc.tensor.matmul(pwarm, lhsT=scratch_bf[:, 0:128],
                                 rhs=scratch_bf, start=(i == 0),
                                 stop=(i == N_WARM - 1))

            # ---------- input DMAs: one HWDGE ring, need-order ----------
            xtb_sb = consts.tile([P, NHC, T], BF)
            xtlo_sb = consts.tile([P, NHC, T], BF)
            gcat_sb = consts.tile([P, NHC, 2 * E], BF)
            biasb_sb = consts.tile([P, E], F32)
            selbc_sb = consts.tile([E, E_LOC * P], BF)
            wg_sb, wu_sb, wd_sb = [], [], []
            for e in range(E_LOC):
                wg_sb.append(wpool.tile([P, 2, NHC, 256], BF, name=f'wg{e}', tag=f'wg{e}'))
                wu_sb.append(wpool.tile([P, 2, NHC, 256], BF, name=f'wu{e}', tag=f'wu{e}'))
                wd_sb.append(wpool.tile([P, NIC, H], BF, name=f'wd{e}', tag=f'wd{e}'))

            WSEG = 2 * NHC * 256          # per-expert elems/partition (wg/wu)
            DSEG = NIC * H                # per-expert elems/partition (wd)

            def dma_gu(w_sb, w_d, e, s=None):
                if s is None:
                    nc.sync.dma_start(
                        w_sb[e].rearrange("p s c i -> p (s c i)"),
                        w_d[:, e * WSEG:(e + 1) * WSEG])
                else:
                    nc.sync.dma_start(
                        w_sb[e][:, s].rearrange("p c i -> p (c i)"),
                        w_d[:, e * WSEG + s * (WSEG // 2):
                            e * WSEG + (s + 1) * (WSEG // 2)])

            def dma_wd(e):
                nc.sync.dma_start(
                    wd_sb[e].rearrange("p c h -> p (c h)"),
                    wd_d[:, e * DSEG:(e + 1) * DSEG])

            def dma_gu2(eng, w_sb, w_d, e, s=None):
                if s is None:
                    eng.dma_start(
                        w_sb[e].rearrange("p s c i -> p (s c i)"),
                        w_d[:, e * WSEG:(e + 1) * WSEG])
                else:
                    eng.dma_start(
                        w_sb[e][:, s].rearrange("p c i -> p (c i)"),
                        w_d[:, e * WSEG + s * (WSEG // 2):
                            e * WSEG + (s + 1) * (WSEG // 2)])

            def dma_wd2(eng, e):
                eng.dma_start(
                    wd_sb[e].rearrange("p c h -> p (c h)"),
                    wd_d[:, e * DSEG:(e + 1) * DSEG])

            # single sync ring, need-order (per-core DMA BW is the
            # binding constraint; a second ring just splits the same BW)
            nc.sync.dma_start(gcat_sb.rearrange("p c e -> p (c e)"), gcat_d[:, :])
            nc.sync.dma_start(xtb_sb.rearrange("p c t -> p (c t)"), xtb_d[:, :])
            nc.sync.dma_start(xtlo_sb.rearrange("p c t -> p (c t)"), xtlo_d[:, :])
            nc.sync.dma_start(biasb_sb, biasb_d[:, :])
            nc.sync.dma_start(selbc_sb, selbc_d[:, :])
            dma_gu2(nc.sync, wg_sb, wg_d, 0, 0)
            dma_gu2(nc.sync, wu_sb, wu_d, 0, 0)
            dma_gu2(nc.sync, wg_sb, wg_d, 0, 1)
            dma_gu2(nc.sync, wu_sb, wu_d, 0, 1)
            dma_gu2(nc.sync, wg_sb, wg_d, 1)
            dma_gu2(nc.sync, wu_sb, wu_d, 1)
            dma_wd2(nc.sync, 0)
            dma_gu2(nc.sync, wg_sb, wg_d, 2)
            dma_gu2(nc.sync, wu_sb, wu_d, 2)
            dma_wd2(nc.sync, 1)
            dma_gu2(nc.sync, wg_sb, wg_d, 3)
            dma_gu2(nc.sync, wu_sb, wu_d, 3)
            dma_wd2(nc.sync, 2)
            dma_wd2(nc.sync, 3)

            # ---------- routing (replicated, split-precision fp16) ----------
            rwT_sb = consts.tile([E, T], F32)
            for tt in range(NTT):
                tsl = slice(tt * P, (tt + 1) * P)
                pl = ps.tile([P, 2 * E], F32, name='pl', tag='ps_small', bufs=2)
                for c in range(NHC):
                    nc.tensor.matmul(pl, lhsT=xtb_sb[:, c, tsl],
                                     rhs=gcat_sb[:, c, :],
                                     start=(c == 0), stop=False)
                for c in range(NHC):
                    nc.tensor.matmul(pl[:, 0:E], lhsT=xtlo_sb[:, c, tsl],
                                     rhs=gcat_sb[:, c, 0:E],
                                     start=False, stop=(c == NHC - 1))
                lhalf = rt.tile([P, E], F32, name='lhalf', tag='lhalf')
                nc.vector.tensor_copy(lhalf, pl[:, E:2 * E])
                lsum = rt.tile([P, E], F32, name='lsum', tag='lsum')
                nc.vector.tensor_add(lsum, pl[:, 0:E], lhalf)
                scores = rt.tile([P, E], F32, name='scores', tag='scores')
                nc.scalar.activation(scores, lsum, Act.Sigmoid)
                s4c = rt.tile([P, E], F32, name='s4c', tag='s4c')
                nc.vector.tensor_add(s4c, scores, biasb_sb)

                # group score: sum of top-2 of each group of 4
                s4c3 = s4c.rearrange("p (g j) -> p g j", j=GSZ)
                v = [s4c3[:, :, j] for j in range(GSZ)]
                m1 = rt.tile([P, N_GROUP], F32, name='m1', tag='m1')
                n1 = rt.tile([P, N_GROUP], F32, name='n1', tag='n1')
                m2 = rt.tile([P, N_GROUP], F32, name='m2', tag='m2')
                n2 = rt.tile([P, N_GROUP], F32, name='n2', tag='n2')
                nc.vector.tensor_tensor(m1, v[0], v[1], op=Alu.max)
                nc.vector.tensor_tensor(n1, v[0], v[1], op=Alu.min)
                nc.vector.tensor_tensor(m2, v[2], v[3], op=Alu.max)
                nc.vector.tensor_tensor(n2, v[2], v[3], op=Alu.min)
                top1 = rt.tile([P, N_GROUP], F32, name='top1', tag='top1')
                mn = rt.tile([P, N_GROUP], F32, name='mn', tag='mn')
                mx2 = rt.tile([P, N_GROUP], F32, name='mx2', tag='mx2')
                sec = rt.tile([P, N_GROUP], F32, name='sec', tag='sec')
                nc.vector.tensor_tensor(top1, m1, m2, op=Alu.max)
                nc.vector.tensor_tensor(mn, m1, m2, op=Alu.min)
                nc.vector.tensor_tensor(mx2, n1, n2, op=Alu.max)
                nc.vector.tensor_tensor(sec, mn, mx2, op=Alu.max)
                gsc = rt.tile([P, N_GROUP], F32, name='gsc', tag='gsc')
                nc.vector.tensor_add(gsc, top1, sec)

                # top-4 groups: threshold at 4th largest group score
                g8 = rt.tile([P, 8], F32, name='g8', tag='g8')
                nc.vector.max(g8, gsc)
                gmask = rt.tile([P, N_GROUP], F32, name='gmask', tag='gmask')
                nc.vector.tensor_scalar(gmask, gsc, g8[:, 3:4], None, op0=Alu.is_ge)

                # masked scores, top-8 experts by threshold
                masked = rt.tile([P, E], F32, name='masked', tag='masked')
                masked3 = masked.rearrange("p (g j) -> p g j", j=GSZ)
                for j in range(GSZ):
                    nc.vector.tensor_tensor(masked3[:, :, j], v[j], gmask,
                                            op=Alu.mult)
                t8 = rt.tile([P, 8], F32, name='t8', tag='t8')
                nc.vector.max(t8, masked)
                selm = rt.tile([P, E], F32, name='selm', tag='selm')
                nc.vector.tensor_scalar(selm, masked, t8[:, 7:8], None,
                                        op0=Alu.is_ge)

                # routing weights: raw scores of selected, normalized, *2.5
                rw_raw = rt.tile([P, E], F32, name='rw_raw', tag='rw_raw')
                nc.vector.tensor_tensor(rw_raw, scores, selm, op=Alu.mult)
                den = rt.tile([P, 1], F32, name='den', tag='den')
                nc.vector.tensor_reduce(den, rw_raw, axis=mybir.AxisListType.X,
                                        op=Alu.add)
                inv = rt.tile([P, 1], F32, name='inv', tag='inv')
                nc.vector.reciprocal(inv, den)
                rw = rt.tile([P, E], F32, name='rw', tag='rw')
                nc.vector.tensor_scalar(rw, rw_raw, inv,
                                        ROUTED_SCALING_FACTOR,
                                        op0=Alu.mult, op1=Alu.mult)

                # transpose [128, 32] -> [32, 128] via DVE 32x32 blocks
                for i in range(4):
                    nc.vector.transpose(
                        rwT_sb[:, tt * P + 32 * i:tt * P + 32 * (i + 1)],
                        rw[32 * i:32 * (i + 1), :])

            # ---------- expert MLP (bf16 matmuls, fp32 accumulate) ----------
            yps = [psy.tile([P, 512], F32, name=f'y{tt}_{hh}', tag=f'y{tt}_{hh}')
                   for tt in range(NTT) for hh in range(HH)]

            atiles = {}
            t1s = {}

            def emit_gu(e):
                for ic in range(NIC):
                    s, k = divmod(ic, 2)
                    icsl = slice(k * P, (k + 1) * P)
                    pgu = ps.tile([P, 2, T], F32, name=f'pgu{e}_{ic}',
                                  tag='ps_gu', bufs=2)
                    pg = pgu[:, 0, :]
                    pu = pgu[:, 1, :]
                    for c in range(NHC):
                        nc.tensor.matmul(pg, lhsT=wg_sb[e][:, s, c, icsl],
                                         rhs=xtb_sb[:, c, :],
                                         start=(c == 0), stop=(c == NHC - 1))
                    for c in range(NHC):
                        nc.tensor.matmul(pu, lhsT=wu_sb[e][:, s, c, icsl],
                                         rhs=xtb_sb[:, c, :],
                                         start=(c == 0), stop=(c == NHC - 1))
                    sg = actp.tile([P, T], F32, name=f'sg{e}_{ic}', tag='sg')
                    nc.scalar.activation(sg, pg, Act.Silu)
                    t1 = actp.tile([P, T], F32, name=f't1{e}_{ic}', tag='t1')
                    nc.vector.tensor_mul(t1, sg, pu)
                    t1s[(e, ic)] = t1

            def emit_at(e):
                for ic in range(NIC):
                    at = atp.tile([P, T], BF, name=f'at{e}_{ic}',
                                  tag=f'at{e}_{ic}')
                    nc.vector.tensor_mul(at, t1s[(e, ic)], rwb_sb[:, e, :])
                    atiles[(e, ic)] = at

            def emit_down(e):
                first = (e == 0)
                last = (e == E_LOC - 1)
                if not last:
                    for ic in range(NIC):
                        at = atiles[(e, ic)]
                        for tt in range(NTT):
                            for hh in range(HH):
                                nc.tensor.matmul(
                                    yps[tt * HH + hh],
                                    lhsT=at[:, tt * P:(tt + 1) * P],
                                    rhs=wd_sb[e][:, ic, hh * 512:(hh + 1) * 512],
                                    start=(first and ic == 0), stop=False)
                else:
                    # tile-major so tt0 PSUM groups close (and drain) early
                    for tt in range(NTT):
                        for hh in range(HH):
                            for ic in range(NIC):
                                nc.tensor.matmul(
                                    yps[tt * HH + hh],
                                    lhsT=atiles[(e, ic)][:, tt * P:(tt + 1) * P],
                                    rhs=wd_sb[e][:, ic, hh * 512:(hh + 1) * 512],
                                    start=False, stop=(ic == NIC - 1))

            rwb_sb = consts.tile([P, E_LOC, T], F32)

            def emit_rwb():
                # split rwT to hi/lo fp16 for exact-ish broadcast matmuls
                rwT_hi = consts.tile([E, T], BF)
                nc.vector.tensor_copy(rwT_hi, rwT_sb)
                rwT_lo = consts.tile([E, T], BF)
                nc.vector.tensor_sub(rwT_lo, rwT_sb, rwT_hi)

                # broadcast local experts' routing weights across partitions
                for j in range(E_LOC):
                    pbj = ps.tile([P, T], F32, name=f'pbj{j}', tag='ps_gu', bufs=2)
                    nc.tensor.matmul(pbj, lhsT=selbc_sb[:, j * P:(j + 1) * P],
                                     rhs=rwT_hi, start=True, stop=False)
                    nc.tensor.matmul(pbj, lhsT=selbc_sb[:, j * P:(j + 1) * P],
                                     rhs=rwT_lo, start=False, stop=True)
                    nc.vector.tensor_copy(rwb_sb[:, j, :], pbj)


            # software-pipeline: expert e's down-proj is emitted after
            # expert e+1's up/gate matmuls so PE never waits on DVE.
            # rwb matmuls sit after gu(0) so PE isn't stuck behind the
            # routing DVE chain.
            emit_gu(0)
            emit_rwb()
            emit_at(0)
            for e in range(1, E_LOC):
                emit_gu(e)
                emit_at(e)
                emit_down(e - 1)
            emit_down(E_LOC - 1)

            # ---------- drain partial output (pipelined, both rings) ---
            for tt in range(NTT):
                tsl = slice(tt * P, (tt + 1) * P)
                osb = outp.tile([P, H], BF, name=f'osb{tt}', tag=f'osb{tt}')
                for hh in range(HH):
                    hsl = slice(hh * 512, (hh + 1) * 512)
                    nc.vector.tensor_copy(osb[:, hsl], yps[tt * HH + hh])
                    nc.sync.dma_start(out_d[tsl, hsl], osb[:, hsl])

    _spill_excess_waits(nc)
    return nc


def _spill_excess_waits(nc, max_waits=1):
    """walrus codegen in this container accepts at most one semaphore wait
    per engine instruction; move extra waits onto preceding same-engine NOPs
    (engine queues are in-order, so this preserves the synchronization)."""
    f = nc.m.functions[0]
    n_spilled = 0
    for b in f.blocks:
        new_insts = []
        for inst in b.instructions:
            si = inst.sync_info
            if si is not None and si.on_wait is not None \
                    and len(si.on_wait) > max_waits:
                waits = list(si.on_wait)
                keep = waits[-max_waits:]
                extra = waits[:-max_waits]
                for k, w in enumerate(extra):
                    nop = mybir.InstNoOp(
                        name=f"{inst.name}-wspill{k}",
                        sync_info=mybir.SyncInfo(on_wait=[w], on_update=[]),
                        bass_nofuse=True,
                        engine=inst.engine,
                    )
                    new_insts.append(nop)
                    n_spilled += 1
                inst.sync_info = mybir.SyncInfo(
                    on_wait=keep, on_update=list(si.on_update or []))
            new_insts.append(inst)
        b.instructions = new_insts


def kernel(x, gate_w, e_score_bias, Wg, Wu, Wd):
    if 'nc' not in _CACHE:
        _CACHE['nc'] = _build()
    nc = _CACHE['nc']

    f16 = np.float16

    def pmajor_ht(a):
        # [H, N] -> [P, NHC*N]: row h = c*128+p goes to (p, c*N + :)
        n = a.shape[1]
        return np.ascontiguousarray(
            a.reshape(NHC, P, n).transpose(1, 0, 2).reshape(P, NHC * n))

    xT = np.ascontiguousarray(np.asarray(x).T).astype(np.float32)
    xTb = xT.astype(f16)
    xTlo = (xT - xTb.astype(np.float32)).astype(f16)
    gate = np.ascontiguousarray(np.asarray(gate_w)).astype(np.float32)
    ghi = gate.astype(f16)
    glo = (gate - ghi.astype(np.float32)).astype(f16)
    gcat = np.concatenate([ghi, glo], axis=1)          # [H, 2E]
    biasb = np.broadcast_to(
        np.asarray(e_score_bias).astype(np.float32)[None, :], (P, E)).copy()
    # weights: wg/wu [p, e, s, c, i'] (i = s*256+i'); wd [p, e, c, h]
    Wgb = np.asarray(Wg).astype(f16).reshape(E, NHC, P, 2, 256)
    Wgb = np.ascontiguousarray(Wgb.transpose(2, 0, 3, 1, 4))   # [P,E,2,NHC,256]
    Wub = np.asarray(Wu).astype(f16).reshape(E, NHC, P, 2, 256)
    Wub = np.ascontiguousarray(Wub.transpose(2, 0, 3, 1, 4))
    Wdb = np.asarray(Wd).astype(f16).reshape(E, NIC, P, H)
    Wdb = np.ascontiguousarray(Wdb.transpose(2, 0, 1, 3))      # [P,E,NIC,H]

    in_maps = []
    for c in range(N_CORES):
        sel = np.zeros((E, E_LOC, P), dtype=f16)
        for j in range(E_LOC):
            sel[c * E_LOC + j, j, :] = 1.0
        esl = slice(c * E_LOC, (c + 1) * E_LOC)
        in_maps.append({
            'xtb': pmajor_ht(xTb),
            'xtlo': pmajor_ht(xTlo),
            'gcat': pmajor_ht(gcat),
            'biasb': biasb,
            'selbc': sel.reshape(E, E_LOC * P),
            'wg': np.ascontiguousarray(Wgb[:, esl]).reshape(P, -1),
            'wu': np.ascontiguousarray(Wub[:, esl]).reshape(P, -1),
            'wd': np.ascontiguousarray(Wdb[:, esl]).reshape(P, -1),
        })

    _CACHE['in_maps'] = in_maps
    res = run_bass_kernel_spmd(nc, in_maps, core_ids=list(range(N_CORES)))
    out = np.zeros((T, H), dtype=np.float32)
    for c in range(N_CORES):
        out += res.results[c]['out'].astype(np.float32)
    return out


def run_traced(**kwargs):
    """Re-run the last kernel invocation with NTFF tracing enabled."""
    return run_bass_kernel_spmd(_CACHE['nc'], _CACHE['in_maps'],
                                core_ids=list(range(N_CORES)), trace=True,
                                **kwargs)



# revision 3
# speedup vs baseline: 1.3060x; 1.3060x over previous
"""Trainium2 Bass kernel for DeepSeek-V3-style block-sparse MoE MLP.

Strategy v2 (host-routed sparse dispatch, expert-parallel on 8 cores):
  - Routing (group-limited top-k) is computed EXACTLY on the host in
    numpy; selection margins for these inputs are >=1e-4, far above f32
    noise, so the selection matches the jax reference bit-for-bit.
  - Each core holds 4 of the 32 experts. The host gathers each expert's
    assigned tokens (<=C capacity slots, zero-padded) so the device does
    sparse compute: only top-8/32 of the dense token*expert work
    (~49k PE rows/core vs ~107k dense), which hides entirely under the
    ~12.6 MB/core fp16 weight DMA (~30 us at ~420 GB/s).
  - Routing weights are folded into the up-projection activations via a
    host-broadcast [128, C] multiplier tile, so the down-projection
    directly yields the weighted partial output per expert.
  - Outputs return per-expert [h, slot] fp16 panels; the host does the
    (data-dependent) scatter-add into the [T, H] f32 result.
  - DMA is one need-ordered sync ring: per-(expert, i-block-pair) weight
    chunks so PE starts ~2 us after the first bytes; the last expert's
    down-weights stream per-i-block and its down-proj accumulates
    ib-outermost, so only ~8 short matmuls trail the final weight byte.
"""
import sys
sys.path.insert(0, '/opt/trn_rl_repo')
import numpy as np
import concourse.mybir as mybir
import concourse.tile as tile
from concourse import bass
from concourse.bass_utils import run_bass_kernel_spmd

T, H, I, E = 256, 1024, 512, 32
N_CORES = 8
E_LOC = E // N_CORES            # 4 experts per core
N_GROUP, GSZ = 8, 4
TOP_K = 8
TOPK_GROUP = 4
ROUTED_SCALING_FACTOR = 2.5
P = 128
NHC = H // P                    # 8 h-chunks (contraction for up/gate)
NIB = I // P                    # 4 i-blocks
NHB = H // P                    # 8 h-blocks (down-proj output)
dt = mybir.dt
F32, F16 = dt.float32, dt.float16
Act = mybir.ActivationFunctionType

_CACHE = {}


def _build(C):
    nc = bass.Bass('TRN2')
    # layouts chosen so every DMA reads long contiguous runs per partition
    xg_d = nc.dram_tensor('xg', [P, E_LOC * NHC * C], F16, kind='ExternalInput')
    wbc_d = nc.dram_tensor('wbc', [P, E_LOC * C], F16, kind='ExternalInput')
    # gu: [p, e, ib, g/u, c, i'], wd: [p, e, ib, h]
    gu_d = nc.dram_tensor('gu', [P, E_LOC * NIB * 2 * NHC * P], F16,
                          kind='ExternalInput')
    wd_d = nc.dram_tensor('wd', [P, E_LOC * NIB * H], F16, kind='ExternalInput')
    out_d = nc.dram_tensor('out', [E_LOC * P, NHB * C], F16,
                           kind='ExternalOutput')

    GUSEG = NIB * 2 * NHC * P       # per-expert elems/partition in gu
    GUIB2 = 2 * 2 * NHC * P         # per-(expert, ib-pair) elems/partition
    WDSEG = NIB * H                 # per-expert elems/partition in wd

    with tile.TileContext(nc) as tc:
        with tc.tile_pool(name='consts', bufs=1) as consts, \
             tc.tile_pool(name='wpool', bufs=1) as wpool, \
             tc.tile_pool(name='actp', bufs=3) as actp, \
             tc.tile_pool(name='atp', bufs=1) as atp, \
             tc.tile_pool(name='outp', bufs=1) as outp, \
             tc.tile_pool(name='ps', bufs=1, space='PSUM') as ps, \
             tc.tile_pool(name='psy', bufs=1, space='PSUM') as psy:

            # ---------- PE warmup (ramps the PE clock during DMA head) ----
            scratch = consts.tile([P, 512], F16)
            nc.vector.memset(scratch, 0.0)
            pwarm = ps.tile([P, 512], F32, name='pwarm', tag='ps_warm', bufs=1)
            N_WARM = 10
            for i in range(N_WARM):
                nc.tensor.matmul(pwarm, lhsT=scratch[:, 0:128],
                                 rhs=scratch, start=(i == 0),
                                 stop=(i == N_WARM - 1))

            # ---------- SBUF tiles ----------
            xg_sb = consts.tile([P, E_LOC, NHC, C], F16)
            wbc_sb = consts.tile([P, E_LOC, C], F16)
            wgu_sb, wd_sb = [], []
            for e in range(E_LOC):
                wgu_sb.append(wpool.tile([P, NIB, 2, NHC, P], F16,
                                         name=f'wgu{e}', tag=f'wgu{e}'))
                wd_sb.append(wpool.tile([P, NIB, H], F16,
                                        name=f'wd{e}', tag=f'wd{e}'))

            def dma_gu(e, half):
                # half = 0 -> ib 0..1, half = 1 -> ib 2..3
                nc.sync.dma_start(
                    wgu_sb[e][:, 2 * half:2 * half + 2].rearrange(
                        "p b t c i -> p (b t c i)"),
                    gu_d[:, e * GUSEG + half * GUIB2:
                         e * GUSEG + (half + 1) * GUIB2])

            def dma_wd(e, ib=None):
                if ib is None:
                    nc.sync.dma_start(
                        wd_sb[e].rearrange("p b h -> p (b h)"),
                        wd_d[:, e * WDSEG:(e + 1) * WDSEG])
                else:
                    nc.sync.dma_start(
                        wd_sb[e][:, ib],
                        wd_d[:, e * WDSEG + ib * H:e * WDSEG + (ib + 1) * H])

            # ---------- input DMAs: one sync ring, need-order ----------
            nc.sync.dma_start(xg_sb[:, 0].rearrange("p c t -> p (c t)"),
                              xg_d[:, 0:NHC * C])
            nc.sync.dma_start(wbc_sb.rearrange("p e t -> p (e t)"),
                              wbc_d[:, :])
            dma_gu(0, 0)
            dma_gu(0, 1)
            nc.sync.dma_start(
                xg_sb[:, 1:E_LOC].rearrange("p e c t -> p (e c t)"),
                xg_d[:, NHC * C:E_LOC * NHC * C])
            dma_wd(0)

            # ---------- per-expert compute ----------
            atiles = {}

            def emit_ug(e):
                for ib in range(NIB):
                    pgu = ps.tile([P, 2, C], F32, name=f'pgu{e}_{ib}',
                                  tag='ps_gu', bufs=2)
                    pg = pgu[:, 0, :]
                    pu = pgu[:, 1, :]
                    for c in range(NHC):
                        nc.tensor.matmul(pg, lhsT=wgu_sb[e][:, ib, 0, c, :],
                                         rhs=xg_sb[:, e, c, :],
                                         start=(c == 0), stop=(c == NHC - 1))
                    for c in range(NHC):
                        nc.tensor.matmul(pu, lhsT=wgu_sb[e][:, ib, 1, c, :],
                                         rhs=xg_sb[:, e, c, :],
                                         start=(c == 0), stop=(c == NHC - 1))
                    # puw = u * routing weight (DVE) in parallel with silu (Act)
                    puw = actp.tile([P, C], F32, name=f'puw{e}_{ib}', tag='puw')
                    nc.vector.tensor_mul(puw, pu, wbc_sb[:, e, :])
                    sg = actp.tile([P, C], F32, name=f'sg{e}_{ib}', tag='sg')
                    nc.scalar.activation(sg, pg, Act.Silu)
                    at = atp.tile([P, C], F16, name=f'at{e}_{ib}',
                                  tag=f'at{e % 2}_{ib}', bufs=1)
                    nc.vector.tensor_mul(at, sg, puw)
                    atiles[(e, ib)] = at

            def emit_down(e):
                yb = [psy.tile([P, 4 * C], F32, name=f'y{e}_{half}',
                               tag=f'ps_y{half}', bufs=2) for half in range(2)]
                # ib outermost: the last wd chunk gates only NHB matmuls.
                # start/stop once per PSUM BANK: start_tensor_calc arms a
                # 2KB zero-region ("zero on next write"), so re-arming per
                # sub-region would discard earlier sub-regions' accumulation.
                for ib in range(NIB):
                    for hb in range(NHB):
                        nc.tensor.matmul(
                            yb[hb // 4][:, (hb % 4) * C:(hb % 4 + 1) * C],
                            lhsT=wd_sb[e][:, ib, hb * P:(hb + 1) * P],
                            rhs=atiles[(e, ib)],
                            start=(ib == 0 and hb % 4 == 0),
                            stop=(ib == NIB - 1 and hb % 4 == 3))
                osb = outp.tile([P, NHB * C], F16, name=f'osb{e}', tag=f'osb{e}')
                nc.vector.tensor_copy(osb[:, 0:4 * C], yb[0])
                nc.vector.tensor_copy(osb[:, 4 * C:8 * C], yb[1])
                return osb

            def dma_out(e, osb):
                nc.sync.dma_start(out_d[e * P:(e + 1) * P, :], osb)

            # software pipeline; sync-ring emission order == transfer order
            emit_ug(0)
            dma_gu(1, 0)
            dma_gu(1, 1)
            osb0 = emit_down(0)
            dma_wd(1)
            emit_ug(1)
            dma_gu(2, 0)
            dma_gu(2, 1)
            dma_out(0, osb0)
            osb1 = emit_down(1)
            dma_wd(2)
            emit_ug(2)
            dma_gu(3, 0)
            dma_gu(3, 1)
            dma_out(1, osb1)
            osb2 = emit_down(2)
            for ib in range(NIB):
                dma_wd(3, ib)
            emit_ug(3)
            dma_out(2, osb2)
            osb3 = emit_down(3)
            dma_out(3, osb3)

    _spill_excess_waits(nc)
    return nc


def _spill_excess_waits(nc, max_waits=1):
    """walrus codegen in this container accepts at most one semaphore wait
    per engine instruction; move extra waits onto preceding same-engine NOPs
    (engine queues are in-order, so this preserves the synchronization)."""
    f = nc.m.functions[0]
    for b in f.blocks:
        new_insts = []
        for inst in b.instructions:
            si = inst.sync_info
            if si is not None and si.on_wait is not None \
                    and len(si.on_wait) > max_waits:
                waits = list(si.on_wait)
                keep = waits[-max_waits:]
                extra = waits[:-max_waits]
                for k, w in enumerate(extra):
                    nop = mybir.InstNoOp(
                        name=f"{inst.name}-wspill{k}",
                        sync_info=mybir.SyncInfo(on_wait=[w], on_update=[]),
                        bass_nofuse=True,
                        engine=inst.engine,
                    )
                    new_insts.append(nop)
                inst.sync_info = mybir.SyncInfo(
                    on_wait=keep, on_update=list(si.on_update or []))
            new_insts.append(inst)
        b.instructions = new_insts


# ---------------- host-side routing (exact numpy replica) ----------------

def _topk_np(a, k):
    # ties broken by lower index, like jax.lax.top_k
    idx = np.argsort(-a, axis=-1, kind='stable')[..., :k]
    return np.take_along_axis(a, idx, axis=-1), idx


def _route_ds3_np(x, gate_w, e_score_bias):
    logits = x.astype(np.float32) @ gate_w.astype(np.float32)
    scores = 1.0 / (1.0 + np.exp(-logits))
    s4c = scores + e_score_bias[None, :].astype(np.float32)
    gsz = E // N_GROUP
    grouped = s4c.reshape(-1, N_GROUP, gsz)
    g2, _ = _topk_np(grouped, 2)
    _, group_idx = _topk_np(g2.sum(-1), TOPK_GROUP)
    group_mask = np.zeros((x.shape[0], N_GROUP), np.float32)
    np.put_along_axis(group_mask, group_idx, 1.0, axis=1)
    masked = np.where(np.repeat(group_mask, gsz, axis=-1) > 0, s4c, 0.0)
    _, topk_idx = _topk_np(masked, TOP_K)
    topk_w = np.take_along_axis(scores, topk_idx, axis=1)
    topk_w = topk_w / (topk_w.sum(-1, keepdims=True) + 1e-20)
    return topk_idx, topk_w * ROUTED_SCALING_FACTOR


def kernel(x, gate_w, e_score_bias, Wg, Wu, Wd):
    f16 = np.float16
    x = np.asarray(x, dtype=np.float32)
    topk_idx, topk_w = _route_ds3_np(
        x, np.asarray(gate_w), np.asarray(e_score_bias))

    # token lists + weights per expert
    toks, ws = [], []
    for e in range(E):
        te, je = np.nonzero(topk_idx == e)
        toks.append(te)
        ws.append(topk_w[te, je].astype(np.float32))
    max_cnt = max(len(t) for t in toks)
    C = 128 if max_cnt <= 128 else int(np.ceil(max_cnt / 32) * 32)

    if _CACHE.get('C') != C:
        _CACHE['C'] = C
        _CACHE['nc'] = _build(C)
    nc = _CACHE['nc']

    # x^T in partition-major layout [p, c, t]
    xTp = np.ascontiguousarray(
        x.T.reshape(NHC, P, T).transpose(1, 0, 2)).astype(f16)  # [P, NHC, T]

    Wg_ = np.asarray(Wg).astype(f16)
    Wu_ = np.asarray(Wu).astype(f16)
    Wd_ = np.asarray(Wd).astype(f16)
    # gu host layout: [e][p, ib, g/u, c, i']
    gu_all = np.empty((E, P, NIB, 2, NHC, P), f16)
    for e in range(E):
        # [H, I] -> [c, h', ib, i'] -> [h', ib, c, i']
        g4 = Wg_[e].reshape(NHC, P, NIB, P).transpose(1, 2, 0, 3)
        u4 = Wu_[e].reshape(NHC, P, NIB, P).transpose(1, 2, 0, 3)
        gu_all[e, :, :, 0] = g4
        gu_all[e, :, :, 1] = u4
    # wd host layout: [e][p, ib, h]  (I = ib*128 + p)
    wd_all = Wd_.reshape(E, NIB, P, H).transpose(0, 2, 1, 3)

    in_maps = []
    for c in range(N_CORES):
        xg = np.zeros((P, E_LOC, NHC, C), f16)
        wbc = np.zeros((P, E_LOC, C), f16)
        for j in range(E_LOC):
            e = c * E_LOC + j
            tl = toks[e]
            xg[:, j, :, :len(tl)] = xTp[:, :, tl]
            wbc[:, j, :len(tl)] = ws[e][None, :].astype(f16)
        esl = slice(c * E_LOC, (c + 1) * E_LOC)
        in_maps.append({
            'xg': np.ascontiguousarray(xg).reshape(P, -1),
            'wbc': np.ascontiguousarray(wbc).reshape(P, -1),
            'gu': np.ascontiguousarray(
                gu_all[esl].transpose(1, 0, 2, 3, 4, 5)).reshape(P, -1),
            'wd': np.ascontiguousarray(
                wd_all[esl].transpose(1, 0, 2, 3)).reshape(P, -1),
        })

    _CACHE['in_maps'] = in_maps
    res = run_bass_kernel_spmd(nc, in_maps, core_ids=list(range(N_CORES)))

    out = np.zeros((T, H), dtype=np.float32)
    for c in range(N_CORES):
        arr = res.results[c]['out'].astype(np.float32)  # [E_LOC*P, NHB*C]
        for j in range(E_LOC):
            e = c * E_LOC + j
            tl = toks[e]
            if len(tl) == 0:
                continue
            # [p, hb, slot] -> [hb, p, slot] -> [h, slot]
            y = arr[j * P:(j + 1) * P].reshape(P, NHB, C).transpose(1, 0, 2)
            out[tl] += y.reshape(H, C)[:, :len(tl)].T
    return out


def run_traced(**kwargs):
    """Re-run the last kernel invocation with NTFF tracing enabled."""
    return run_bass_kernel_spmd(_CACHE['nc'], _CACHE['in_maps'],
                                core_ids=list(range(N_CORES)), trace=True,
                                **kwargs)
